# revision 26
# baseline (speedup 1.0000x reference)
"""PointConv2 Trainium2 Bass kernel.

Data-parallel over B=8 across 8 NeuronCores (one batch element per core).

Per-core computation (feature F [64,16384] f32, idx [16384,8] i32 -> out [128,16384] f32):
  G  = relu(w2b @ relu(w2a @ F + b2a) + b2b)                 [64, N]
  gf[k*64+c, n]  = F[c, idx[n,k]]
  Gg[k*64+c', n] = G[c', idx[n,k]]
  L  = relu(w1b @ relu(w1a @ gf + b1a) + b1b)                [512, N]
  out = relu(w3b @ relu(w3a @ (Gg + L) + b3a) + b3b)         [128, N]

Strategy (v2 — multi-queue gathers, j-major contiguous layout):
  * All matmuls bf16 (fp32 PSUM accumulate).
  * Phase 1: cast feature to bf16 into FG[0:64], compute G into FG[64:128]
    (matmul col-positioned at partitions 64-127), PE-transpose FG into a
    token-major SBUF table TOK (one 384B row per point: [F|G|F]).
  * Phase 1b: PE-transpose FG and stream it to a DRAM token table TOKD
    [N, 128] bf16 (one 256B [F|G] row per token).
  * Phase 2: per chunk of 1024 points, two dma_gather(transpose=False,
    DRAM-source) calls fetch the 8 neighbor rows per point (4 even k's and
    4 odd k's).  Gathers are issued round-robin over 4 SWDGE queues with
    several chunks in flight; the Q7 cluster generates descriptors for 4
    gathers concurrently (~2ns/item vs ~8ns single-queue; the transposed
    SBUF-source gather corrupts data when run multi-queue, the row-gather
    does not).  A PE transpose stage then re-blocks the gathered rows to
    channel-major columns, swapping halves for even k so odd-k F lands on
    partitions 0-63 and even-k F on 64-127 (balanced PE row groups, and
    partition-aligned Gg+L adds).
  * Gather idx lists are pre-arranged j-major: within a chunk, output
    column i = j*1024 + q (j = k-pair, q = ti*16 + p enumerates points
    n = c*1024 + p*64 + ti).  All layer-1a/1b/3a matmul rhs operands and
    the Gg+L adds are then fully contiguous 512-column slices.  The final
    3b matmul un-permutes q -> n via a strided rhs access pattern so the
    output DMA is contiguous.
"""

import os

import numpy as np

STAGE = int(os.environ.get("PC_STAGE", "9"))
QMULTI = int(os.environ.get("PC_QMULTI", "1"))

N = 16384
P = 128
CH = 1024          # points per gather chunk
NCH = N // CH      # 16
NIDX = CH * 4      # idxs per gather call (4 even or 4 odd k's)
ROWB = 384         # bytes per token row in TOK ([F|G|F] bf16)
GBUF = 4           # gather tiles in flight per parity
TBUF = 3           # transposed tiles in flight per parity

_cache = {}


def _build():
    if "nc" in _cache:
        return _cache["nc"]

    import concourse.bass as bass
    import concourse.mybir as mybir
    import concourse.tile as tile
    from concourse.bacc import Bacc
    from concourse.masks import make_identity

    f32 = mybir.dt.float32
    bf16 = mybir.dt.bfloat16
    i32 = mybir.dt.int32
    i16 = mybir.dt.int16
    RELU = mybir.ActivationFunctionType.Relu
    ADD = mybir.AluOpType.add

    nc = Bacc("TRN2", target_bir_lowering=False, debug=False, num_devices=8,
              num_swdge_queues=4)

    feature = nc.dram_tensor("feature", [64, N], f32, kind="ExternalInput")
    idx = nc.dram_tensor("idx", [N, 8], i32, kind="ExternalInput")
    w1a = nc.dram_tensor("w1a", [512, 512], f32, kind="ExternalInput")
    b1a = nc.dram_tensor("b1a", [512], f32, kind="ExternalInput")
    w1b = nc.dram_tensor("w1b", [512, 512], f32, kind="ExternalInput")
    b1b = nc.dram_tensor("b1b", [512], f32, kind="ExternalInput")
    w2a = nc.dram_tensor("w2a", [128, 64], f32, kind="ExternalInput")
    b2a = nc.dram_tensor("b2a", [128], f32, kind="ExternalInput")
    w2b = nc.dram_tensor("w2b", [64, 128], f32, kind="ExternalInput")
    b2b = nc.dram_tensor("b2b", [64], f32, kind="ExternalInput")
    w3a = nc.dram_tensor("w3a", [256, 512], f32, kind="ExternalInput")
    b3a = nc.dram_tensor("b3a", [256], f32, kind="ExternalInput")
    w3b = nc.dram_tensor("w3b", [128, 256], f32, kind="ExternalInput")
    b3b = nc.dram_tensor("b3b", [128], f32, kind="ExternalInput")
    out = nc.dram_tensor("out", [128, N], f32, kind="ExternalOutput")
    tokd = nc.dram_tensor("tokd", [N, P], bf16, kind="Internal")
    tokd2 = nc.dram_tensor("tokd2", [N, P], bf16, kind="Internal")

    with tile.TileContext(nc) as tc:
        with (
            tc.tile_pool(name="const", bufs=1) as const,
            tc.tile_pool(name="tok", bufs=1) as tokp,
            tc.tile_pool(name="idxp", bufs=1) as idxp,
            tc.tile_pool(name="psum", bufs=8, space="PSUM") as psum,
        ):
            idf = const.tile([P, P], f32)
            make_identity(nc, idf)
            idb = const.tile([P, P], bf16)
            make_identity(nc, idb)

            # ---- weights to bf16 lhsT layouts (via PE transpose) ----
            W1 = const.tile([P, 2048], bf16)    # [c][j*512+m]: p<64 -> w1a[m,(2j+1)*64+p], p>=64 -> w1a[m,(2j)*64+(p-64)]
            # zero-padded K=128 variants: odd-k weights (rows 0-63 live) and
            # even-k weights (rows 64-127 live)
            W1Z = const.tile([P, 4096], bf16)
            W1B = const.tile([P, 2048], bf16)   # [p][ci*512+m] = w1b[m, ci*128+p]
            W3A = const.tile([P, 1024], bf16)   # [p][ti*256+m] = w3a[m, ti*128+p]
            W3B = const.tile([P, 256], bf16)    # [p][ti*128+m] = w3b[m, ti*128+p]
            W2A = const.tile([P, 128], bf16)    # [c][m] = w2a[m, c] on partitions 0-63
            W2B = const.tile([P, 64], bf16)     # [p][m] = w2b[m, p]

            with tc.tile_pool(name="wtmp", bufs=2) as wtmp:
                # feature loads as f32 via HWDGE (fast), cast to bf16 on
                # vector/scalar per quarter (overlaps G-mlp pipeline)
                FG = wtmp.tile([P, N], bf16, tag="fg", bufs=1)
                FT32 = wtmp.tile([64, N], f32, tag="ft32", bufs=1)
                for q in range(4):
                    nc.scalar.dma_start(FT32[:, q * 4096 : (q + 1) * 4096],
                                        feature.ap()[:, q * 4096 : (q + 1) * 4096])


                # ---- small W2 preps first: G-mlp + token tables are the
                # critical chain to the first gather; big weight preps follow
                # and overlap the gather warm-up ----
                nat5 = wtmp.tile([P, 64], f32, tag="wnat5", bufs=1)
                nc.sync.dma_start(nat5, w2a.ap())
                nat6 = wtmp.tile([64, 128], f32, tag="wnat6", bufs=1)
                nc.sync.dma_start(nat6, w2b.ap())
                B2A = const.tile([P, 1], f32)
                nc.sync.dma_start(B2A, b2a.ap()[:, None])
                B2B = const.tile([P, 1], f32)
                nc.sync.dma_start(B2B[64:128, :], b2b.ap()[:, None])
                pt = psum.tile([P, P], f32, tag="mm")
                nc.tensor.transpose(pt[0:64, :], nat5, idf)
                nc.vector.tensor_copy(W2A[0:64, :], pt[0:64, :])
                pt = psum.tile([P, P], f32, tag="mm")
                nc.tensor.transpose(pt[:, 0:64], nat6, idf[0:64, 0:64])
                nc.vector.tensor_copy(W2B, pt[:, 0:64])

                # ---- idx prep (j-major wrapped lists) ----
                # L32[p][c][(ti k)] = idx[c*1024 + p*64 + ti, k]; 2KB runs.
                # Within-chunk gather column becomes i = j*1024 + q with
                # q = ti*16 + p <-> point n = c*1024 + p*64 + ti.
                L32 = wtmp.tile([16, 16, 512], i32, tag="i32", bufs=1)
                nc.sync.dma_start(
                    L32, idx.ap().rearrange("(c p ti) k -> p c (ti k)", c=16, p=16)
                )
                I16E = idxp.tile([P, 4096], i16)
                I16O = idxp.tile([P, 4096], i16)
                bit = L32[:].bitcast(i16)  # [16, 16, 1024 (ti k two)]
                # I16E[p][c*256 + j*64 + ti] = bit[p][c][ti*16 + 4j]
                bitv = bit.rearrange("p c (ti sixteen) -> p c sixteen ti", sixteen=16)
                nc.vector.tensor_copy(
                    I16E[0:16, :].rearrange("p (c j m) -> p c j m", c=16, j=4),
                    bitv[:, :, 0:16:4, :],
                )
                nc.vector.tensor_copy(
                    I16O[0:16, :].rearrange("p (c j m) -> p c j m", c=16, j=4),
                    bitv[:, :, 2:16:4, :],
                )
                for g in range(1, 8):
                    nc.sync.dma_start(I16E[16 * g : 16 * (g + 1), :], I16E[0:16, :])
                    nc.sync.dma_start(I16O[16 * g : 16 * (g + 1), :], I16O[0:16, :])

                # ---- phase 1: G = mlp2(F) into FG[64:128], then per-quarter
                # XBAR transpose to token-major and DMA out to the DRAM tables
                # (quarter pipelining overlaps G-mlp, transpose and writes) ----
                tok_v = tokd.ap().rearrange("(r p) e -> p r e", p=P)
                tok2_v = tokd2.ap().rearrange("(r p) e -> p r e", p=P)
                for q in range(8):
                    qs = slice(q * 2048, q * 2048 + 1024)
                    qs2 = slice(q * 2048 + 1024, q * 2048 + 2048)
                    nc.vector.tensor_copy(FG[0:64, qs], FT32[:, qs])
                    nc.scalar.copy(FG[0:64, qs2], FT32[:, qs2])
                    for nt4 in range(4):
                        nt = q * 4 + nt4
                        cols = slice(nt * 512, nt * 512 + 512)
                        g2a_ps = psum.tile([P, 512], f32, tag="mm")
                        nc.tensor.matmul(g2a_ps, W2A[0:64, :], FG[0:64, cols], start=True, stop=True)
                        g2a = wtmp.tile([P, 512], bf16, tag="g2a", bufs=3)
                        nc.scalar.activation(g2a, g2a_ps, RELU, bias=B2A)
                        g2b_ps = psum.tile([P, 512], f32, tag="mm")
                        nc.tensor.matmul(
                            g2b_ps[64:128, :], W2B, g2a, start=True, stop=True,
                            tile_position=(0, 64),
                        )
                        nc.scalar.activation(FG[64:128, cols], g2b_ps[64:128, :], RELU, bias=B2B[64:128, :])
                    STG = wtmp.tile([P, 16, P], bf16, tag="stg", bufs=3, name=f"stg{q}")
                    nc.sync.dma_start_transpose(STG, FG[:, q * 2048 : (q + 1) * 2048])
                    rs = slice(q * 16, q * 16 + 16)
                    eng = nc.sync if q % 2 == 0 else nc.scalar
                    eng2 = nc.scalar if q % 2 == 0 else nc.sync
                    eng.dma_start(tok_v[:, rs, :], STG)
                    eng2.dma_start(tok2_v[:, rs, 0:64], STG[:, :, 64:128])
                    eng2.dma_start(tok2_v[:, rs, 64:128], STG[:, :, 0:64])

                # ---- big weight preps (overlap the gather warm-up) ----
                nat1 = wtmp.tile([P, 4, 512], f32, tag="wnat")
                nc.sync.dma_start(nat1, w1a.ap().rearrange("(ro p) c -> p ro c", p=P))
                for co in range(4):
                    for ro in range(4):
                        pt = psum.tile([P, P], f32, tag="mm")
                        base = co * 128
                        nc.tensor.transpose(pt[0:64, :], nat1[:, ro, base + 64 : base + 128], idf)
                        # transpose-mode MMs must write PSUM partition 0; use a
                        # plain matmul against identity for the partition-64 half
                        nc.tensor.matmul(
                            pt[64:128, :], nat1[:, ro, base : base + 64], idf,
                            start=True, stop=True, tile_position=(0, 64),
                        )
                        nc.vector.tensor_copy(W1[:, co * 512 + ro * 128 : co * 512 + ro * 128 + 128], pt)

                nc.gpsimd.memset(W1Z, 0.0)
                nc.vector.tensor_copy(W1Z[0:64, 0:2048], W1[0:64, :])
                nc.vector.tensor_copy(W1Z[64:128, 2048:4096], W1[64:128, :])

                nat2 = wtmp.tile([P, 4, 512], f32, tag="wnat")
                nc.sync.dma_start(nat2, w1b.ap().rearrange("(ro p) c -> p ro c", p=P))
                for ci in range(4):
                    for mo in range(4):
                        pt = psum.tile([P, P], f32, tag="mm")
                        nc.tensor.transpose(pt, nat2[:, mo, ci * 128 : ci * 128 + 128], idf)
                        nc.vector.tensor_copy(W1B[:, ci * 512 + mo * 128 : ci * 512 + mo * 128 + 128], pt)

                nat3 = wtmp.tile([P, 2, 512], f32, tag="wnat3")
                nc.sync.dma_start(nat3, w3a.ap().rearrange("(ro p) c -> p ro c", p=P))
                for ti in range(4):
                    for mo in range(2):
                        pt = psum.tile([P, P], f32, tag="mm")
                        nc.tensor.transpose(pt, nat3[:, mo, ti * 128 : ti * 128 + 128], idf)
                        nc.vector.tensor_copy(W3A[:, ti * 256 + mo * 128 : ti * 256 + mo * 128 + 128], pt)

                nat4 = wtmp.tile([P, 256], f32, tag="wnat")
                nc.sync.dma_start(nat4, w3b.ap())
                for ti in range(2):
                    pt = psum.tile([P, P], f32, tag="mm")
                    nc.tensor.transpose(pt, nat4[:, ti * 128 : ti * 128 + 128], idf)
                    nc.vector.tensor_copy(W3B[:, ti * 128 : ti * 128 + 128], pt)

                # ---- remaining biases ----
                B1A = const.tile([P, 4], f32)
                nc.sync.dma_start(B1A, b1a.ap().rearrange("(o p) -> p o", p=P))
                B1B = const.tile([P, 4], f32)
                nc.sync.dma_start(B1B, b1b.ap().rearrange("(o p) -> p o", p=P))
                B3A = const.tile([P, 2], f32)
                nc.sync.dma_start(B3A, b3a.ap().rearrange("(o p) -> p o", p=P))
                B3B = const.tile([P, 1], f32)
                nc.sync.dma_start(B3B, b3b.ap()[:, None])

            # ---- phase 2 ----
            with (
                tc.tile_pool(name="gath", bufs=1) as gathp,
                tc.tile_pool(name="work", bufs=1) as workp,
            ):
              gte, gto, tte, tto = {}, {}, {}, {}
              _ordinal = [1]  # FG... pool-DMA ordinal 0 is the W1Z memset? no:
              # ordinal 0 is unused now (memset is not a DMA); first gather
              # ordinal starts at 1 to keep lane/queue alignment with the
              # 8-lane DMASW rotation (queue must equal ordinal mod 4).

              def _gather(dst, tab, idx_ap, n):
                  q = _ordinal[0] % 4 if QMULTI else 0
                  _ordinal[0] += 1
                  nc.gpsimd.dma_gather(
                      dst, tab.ap(), idx_ap,
                      num_idxs=n, num_idxs_reg=n, elem_size=P,
                      transpose=False, single_packet=False, queue_num=q,
                  )

              def issue_gathers(c):
                  gte[c] = gathp.tile([P, 32, P], bf16, tag="gte", bufs=GBUF, name=f"gte{c}")
                  gto[c] = gathp.tile([P, 32, P], bf16, tag="gto", bufs=GBUF, name=f"gto{c}")
                  if c < 3:
                      # warm-up: split into halves so 4 queue streams fill
                      # immediately and the first TT tiles arrive sooner
                      for hh in range(2):
                          _gather(gte[c][:, 16 * hh : 16 * hh + 16, :],
                                  tokd2, I16E[:, c * 256 + 128 * hh : c * 256 + 128 * hh + 128],
                                  NIDX // 2)
                      for hh in range(2):
                          _gather(gto[c][:, 16 * hh : 16 * hh + 16, :],
                                  tokd, I16O[:, c * 256 + 128 * hh : c * 256 + 128 * hh + 128],
                                  NIDX // 2)
                  else:
                      _gather(gte[c][:], tokd2, I16E[:, c * 256 : (c + 1) * 256], NIDX)
                      _gather(gto[c][:], tokd, I16O[:, c * 256 : (c + 1) * 256], NIDX)

              def issue_transposes(c):
                  # PE re-block to channel-major: TT[:, s, p] = GT[p, s, :];
                  # even rows are [G|F] from tokd2 so no parity swap is needed.
                  tte[c] = gathp.tile([P, 32, P], bf16, tag="tte", bufs=TBUF, name=f"tte{c}")
                  tto[c] = gathp.tile([P, 32, P], bf16, tag="tto", bufs=TBUF, name=f"tto{c}")
                  for sb in range(8):
                      pe_ = psum.tile([P, 512], bf16, tag="mm")
                      po_ = psum.tile([P, 512], bf16, tag="mm")
                      for u in range(4):
                          s_ = sb * 4 + u
                          us = slice(u * P, u * P + P)
                          nc.tensor.transpose(pe_[:, us], gte[c][:, s_, :], idb)
                          nc.tensor.transpose(po_[:, us], gto[c][:, s_, :], idb)
                      bs = slice(sb * 4, sb * 4 + 4)
                      if sb % 2 == 0:
                          nc.scalar.copy(tte[c][:, bs, :].rearrange("p a b -> p (a b)"), pe_)
                          nc.vector.tensor_copy(tto[c][:, bs, :].rearrange("p a b -> p (a b)"), po_)
                      else:
                          nc.vector.tensor_copy(tte[c][:, bs, :].rearrange("p a b -> p (a b)"), pe_)
                          nc.scalar.copy(tto[c][:, bs, :].rearrange("p a b -> p (a b)"), po_)

              def issue_compute(c):
                  # within-chunk column i = j*1024 + q, q = ti*16 + p16
                  #   <-> point n = c*1024 + p16*64 + ti
                  # TTE columns are [G|F] (even), TTO are [F|G] (odd).
                  GEv = tte[c].rearrange("p a b -> p (a b)")
                  GOv = tto[c].rearrange("p a b -> p (a b)")

                  # layer 1a for both halves first, so the TT tiles are fully
                  # consumed early and the next chunk's transposes can overlap
                  # the 1b/3a/3b tail.
                  zr = {}
                  for h in range(2):
                      for o in range(4):
                          z1 = psum.tile([P, 512], f32, tag="mm")
                          for j in range(4):
                              cs = slice(j * 1024 + h * 512, j * 1024 + h * 512 + 512)
                              nc.tensor.matmul(
                                  z1, W1Z[:, j * 512 + o * 128 : j * 512 + o * 128 + 128],
                                  GOv[:, cs], start=(j == 0), stop=False,
                              )
                              nc.tensor.matmul(
                                  z1, W1Z[:, 2048 + j * 512 + o * 128 : 2048 + j * 512 + o * 128 + 128],
                                  GEv[:, cs], start=False, stop=(j == 3),
                              )
                          t = workp.tile([P, 512], bf16, tag="zr", bufs=10, name=f"zr{h}{o}")
                          nc.scalar.activation(t, z1, RELU, bias=B1A[:, o : o + 1])
                          zr[h, o] = t
                  # S = Gg + L ... but L is not ready yet; Gg slices are read
                  # into S-staging via the adds below after 1b produces L.
                  lr = {}
                  for h in range(2):
                      for o in range(4):
                          lps = psum.tile([P, 512], f32, tag="mm")
                          for ci in range(4):
                              nc.tensor.matmul(
                                  lps, W1B[:, ci * 512 + o * 128 : ci * 512 + o * 128 + 128],
                                  zr[h, ci], start=(ci == 0), stop=(ci == 3),
                              )
                          t = workp.tile([P, 512], bf16, tag="lr", bufs=10, name=f"lr{h}{o}")
                          nc.scalar.activation(t, lps, RELU, bias=B1B[:, o : o + 1])
                          lr[h, o] = t
                  # S = Gg + L  (even-k G sits on TTE[64:128]: cross-partition)
                  S = {}
                  for h in range(2):
                      for t_ in range(4):
                          cs = slice(t_ * 1024 + h * 512, t_ * 1024 + h * 512 + 512)
                          st = workp.tile([P, 512], bf16, tag="s", bufs=10, name=f"s{h}{t_}")
                          nc.vector.tensor_tensor(
                              st[0:64], GEv[0:64, cs], lr[h, t_][0:64], ADD
                          )
                          nc.vector.tensor_tensor(
                              st[64:128], GOv[64:128, cs], lr[h, t_][64:128], ADD
                          )
                          S[h, t_] = st
                  zr3 = workp.tile([P, 2, CH], bf16, tag="zr3", bufs=3)
                  for h in range(2):
                      hs = slice(h * 512, h * 512 + 512)
                      for o3 in range(2):
                          z3 = psum.tile([P, 512], f32, tag="mm")
                          for t_ in range(4):
                              nc.tensor.matmul(
                                  z3, W3A[:, t_ * 256 + o3 * 128 : t_ * 256 + o3 * 128 + 128],
                                  S[h, t_], start=(t_ == 0), stop=(t_ == 3),
                              )
                          nc.scalar.activation(
                              zr3[:, o3, hs], z3, RELU, bias=B3A[:, o3 : o3 + 1],
                          )
                  # layer 3b: un-permute q = ti*16 + p16 -> n = p16*64 + ti via rhs AP
                  zr3v = zr3.rearrange("p t (ti sixteen) -> p t sixteen ti", sixteen=16)
                  for v in range(2):
                      ops = psum.tile([P, 512], f32, tag="mm")
                      for t_ in range(2):
                          rhs = zr3v[:, t_, 8 * v : 8 * v + 8, :]
                          nc.tensor.matmul(
                              ops, W3B[:, t_ * 128 : t_ * 128 + 128], rhs,
                              start=(t_ == 0), stop=(t_ == 1),
                          )
                      osb = workp.tile([P, 512], f32, tag="osb", bufs=3, name=f"osb{v}")
                      nc.scalar.activation(osb, ops, RELU, bias=B3B)
                      nc.scalar.dma_start(out.ap()[:, c * 1024 + v * 512 : c * 1024 + v * 512 + 512], osb)

              for c in range(min(GBUF, NCH)):
                  issue_gathers(c)
              for c in range(min(TBUF - 1, NCH)):
                  issue_transposes(c)
              for c in range(NCH):
                  issue_compute(c)
                  if c + TBUF - 1 < NCH:
                      issue_transposes(c + TBUF - 1)
                  if c + GBUF < NCH:
                      issue_gathers(c + GBUF)

    nc.compile()
    _cache["nc"] = nc
    return nc


def kernel(**inputs):
    from concourse import bass_utils

    nc = _build()
    feature = np.ascontiguousarray(inputs["feature"], dtype=np.float32)
    idx = np.ascontiguousarray(inputs["idx"], dtype=np.int32)
    weights = {
        k: np.ascontiguousarray(np.asarray(inputs[k]), dtype=np.float32)
        for k in ("w1a", "b1a", "w1b", "b1b", "w2a", "b2a", "w2b", "b2b",
                  "w3a", "b3a", "w3b", "b3b")
    }
    in_maps = []
    for b in range(8):
        m = {"feature": feature[b], "idx": idx[b]}
        m.update(weights)
        in_maps.append(m)
    res = bass_utils.run_bass_kernel_spmd(nc, in_maps, core_ids=list(range(8)))
    return np.stack([res.results[b]["out"] for b in range(8)]).astype(np.float32)


# revision 27
# speedup vs baseline: 1.0042x; 1.0042x over previous
"""PointConv2 Trainium2 Bass kernel.

Data-parallel over B=8 across 8 NeuronCores (one batch element per core).

Per-core computation (feature F [64,16384] f32, idx [16384,8] i32 -> out [128,16384] f32):
  G  = relu(w2b @ relu(w2a @ F + b2a) + b2b)                 [64, N]
  gf[k*64+c, n]  = F[c, idx[n,k]]
  Gg[k*64+c', n] = G[c', idx[n,k]]
  L  = relu(w1b @ relu(w1a @ gf + b1a) + b1b)                [512, N]
  out = relu(w3b @ relu(w3a @ (Gg + L) + b3a) + b3b)         [128, N]

Strategy (v2 — multi-queue gathers, j-major contiguous layout):
  * All matmuls bf16 (fp32 PSUM accumulate).
  * Phase 1: cast feature to bf16 into FG[0:64], compute G into FG[64:128]
    (matmul col-positioned at partitions 64-127), PE-transpose FG into a
    token-major SBUF table TOK (one 384B row per point: [F|G|F]).
  * Phase 1b: PE-transpose FG and stream it to a DRAM token table TOKD
    [N, 128] bf16 (one 256B [F|G] row per token).
  * Phase 2: per chunk of 1024 points, two dma_gather(transpose=False,
    DRAM-source) calls fetch the 8 neighbor rows per point (4 even k's and
    4 odd k's).  Gathers are issued round-robin over 4 SWDGE queues with
    several chunks in flight; the Q7 cluster generates descriptors for 4
    gathers concurrently (~2ns/item vs ~8ns single-queue; the transposed
    SBUF-source gather corrupts data when run multi-queue, the row-gather
    does not).  A PE transpose stage then re-blocks the gathered rows to
    channel-major columns, swapping halves for even k so odd-k F lands on
    partitions 0-63 and even-k F on 64-127 (balanced PE row groups, and
    partition-aligned Gg+L adds).
  * Gather idx lists are pre-arranged j-major: within a chunk, output
    column i = j*1024 + q (j = k-pair, q = ti*16 + p enumerates points
    n = c*1024 + p*64 + ti).  All layer-1a/1b/3a matmul rhs operands and
    the Gg+L adds are then fully contiguous 512-column slices.  The final
    3b matmul un-permutes q -> n via a strided rhs access pattern so the
    output DMA is contiguous.
"""

import os

import numpy as np

STAGE = int(os.environ.get("PC_STAGE", "9"))
QMULTI = int(os.environ.get("PC_QMULTI", "1"))

N = 16384
P = 128
CH = 1024          # points per gather chunk
NCH = N // CH      # 16
NIDX = CH * 4      # idxs per gather call (4 even or 4 odd k's)
ROWB = 384         # bytes per token row in TOK ([F|G|F] bf16)
GBUF = 4           # gather tiles in flight per parity
TBUF = 3           # transposed tiles in flight per parity

_cache = {}


def _build():
    if "nc" in _cache:
        return _cache["nc"]

    import concourse.bass as bass
    import concourse.mybir as mybir
    import concourse.tile as tile
    from concourse.bacc import Bacc
    from concourse.masks import make_identity

    f32 = mybir.dt.float32
    bf16 = mybir.dt.bfloat16
    i32 = mybir.dt.int32
    i16 = mybir.dt.int16
    RELU = mybir.ActivationFunctionType.Relu
    ADD = mybir.AluOpType.add

    nc = Bacc("TRN2", target_bir_lowering=False, debug=False, num_devices=8,
              num_swdge_queues=4)

    feature = nc.dram_tensor("feature", [64, N], f32, kind="ExternalInput")
    idx = nc.dram_tensor("idx", [N, 8], i32, kind="ExternalInput")
    w1a = nc.dram_tensor("w1a", [512, 512], f32, kind="ExternalInput")
    b1a = nc.dram_tensor("b1a", [512], f32, kind="ExternalInput")
    w1b = nc.dram_tensor("w1b", [512, 512], f32, kind="ExternalInput")
    b1b = nc.dram_tensor("b1b", [512], f32, kind="ExternalInput")
    w2a = nc.dram_tensor("w2a", [128, 64], f32, kind="ExternalInput")
    b2a = nc.dram_tensor("b2a", [128], f32, kind="ExternalInput")
    w2b = nc.dram_tensor("w2b", [64, 128], f32, kind="ExternalInput")
    b2b = nc.dram_tensor("b2b", [64], f32, kind="ExternalInput")
    w3a = nc.dram_tensor("w3a", [256, 512], f32, kind="ExternalInput")
    b3a = nc.dram_tensor("b3a", [256], f32, kind="ExternalInput")
    w3b = nc.dram_tensor("w3b", [128, 256], f32, kind="ExternalInput")
    b3b = nc.dram_tensor("b3b", [128], f32, kind="ExternalInput")
    out = nc.dram_tensor("out", [128, N], f32, kind="ExternalOutput")
    tokd = nc.dram_tensor("tokd", [N, P], bf16, kind="Internal")
    tokd2 = nc.dram_tensor("tokd2", [N, P], bf16, kind="Internal")

    with tile.TileContext(nc) as tc:
        with (
            tc.tile_pool(name="const", bufs=1) as const,
            tc.tile_pool(name="tok", bufs=1) as tokp,
            tc.tile_pool(name="idxp", bufs=1) as idxp,
            tc.tile_pool(name="psum", bufs=8, space="PSUM") as psum,
        ):
            idf = const.tile([P, P], f32)
            make_identity(nc, idf)
            idb = const.tile([P, P], bf16)
            make_identity(nc, idb)

            # ---- weights to bf16 lhsT layouts (via PE transpose) ----
            W1 = const.tile([P, 2048], bf16)    # [c][j*512+m]: p<64 -> w1a[m,(2j+1)*64+p], p>=64 -> w1a[m,(2j)*64+(p-64)]
            # zero-padded K=128 variants: odd-k weights (rows 0-63 live) and
            # even-k weights (rows 64-127 live)
            W1Z = const.tile([P, 4096], bf16)
            W1B = const.tile([P, 2048], bf16)   # [p][ci*512+m] = w1b[m, ci*128+p]
            W3A = const.tile([P, 1024], bf16)   # [p][ti*256+m] = w3a[m, ti*128+p]
            W3B = const.tile([P, 256], bf16)    # [p][ti*128+m] = w3b[m, ti*128+p]
            W2A = const.tile([P, 128], bf16)    # [c][m] = w2a[m, c] on partitions 0-63
            W2B = const.tile([P, 64], bf16)     # [p][m] = w2b[m, p]

            with tc.tile_pool(name="wtmp", bufs=2) as wtmp:
                # feature loads as f32 via HWDGE (fast), cast to bf16 on
                # vector/scalar per quarter (overlaps G-mlp pipeline)
                FG = wtmp.tile([P, N], bf16, tag="fg", bufs=1)
                FT32 = wtmp.tile([64, N], f32, tag="ft32", bufs=1)
                for q in range(4):
                    nc.scalar.dma_start(FT32[:, q * 4096 : (q + 1) * 4096],
                                        feature.ap()[:, q * 4096 : (q + 1) * 4096])


                # ---- small W2 preps first: G-mlp + token tables are the
                # critical chain to the first gather; big weight preps follow
                # and overlap the gather warm-up ----
                nat5 = wtmp.tile([P, 64], f32, tag="wnat5", bufs=1)
                nc.sync.dma_start(nat5, w2a.ap())
                nat6 = wtmp.tile([64, 128], f32, tag="wnat6", bufs=1)
                nc.sync.dma_start(nat6, w2b.ap())
                B2A = const.tile([P, 1], f32)
                nc.sync.dma_start(B2A, b2a.ap()[:, None])
                B2B = const.tile([P, 1], f32)
                nc.sync.dma_start(B2B[64:128, :], b2b.ap()[:, None])
                pt = psum.tile([P, P], f32, tag="mm")
                nc.tensor.transpose(pt[0:64, :], nat5, idf)
                nc.vector.tensor_copy(W2A[0:64, :], pt[0:64, :])
                pt = psum.tile([P, P], f32, tag="mm")
                nc.tensor.transpose(pt[:, 0:64], nat6, idf[0:64, 0:64])
                nc.vector.tensor_copy(W2B, pt[:, 0:64])

                # ---- idx prep (j-major wrapped lists) ----
                # L32[p][c][(ti k)] = idx[c*1024 + p*64 + ti, k]; 2KB runs.
                # Within-chunk gather column becomes i = j*1024 + q with
                # q = ti*16 + p <-> point n = c*1024 + p*64 + ti.
                L32 = wtmp.tile([16, 16, 512], i32, tag="i32", bufs=1)
                nc.sync.dma_start(
                    L32, idx.ap().rearrange("(c p ti) k -> p c (ti k)", c=16, p=16)
                )
                I16E = idxp.tile([P, 4096], i16)
                I16O = idxp.tile([P, 4096], i16)
                bit = L32[:].bitcast(i16)  # [16, 16, 1024 (ti k two)]
                # I16E[p][c*256 + j*64 + ti] = bit[p][c][ti*16 + 4j]
                bitv = bit.rearrange("p c (ti sixteen) -> p c sixteen ti", sixteen=16)
                nc.vector.tensor_copy(
                    I16E[0:16, :].rearrange("p (c j m) -> p c j m", c=16, j=4),
                    bitv[:, :, 0:16:4, :],
                )
                nc.vector.tensor_copy(
                    I16O[0:16, :].rearrange("p (c j m) -> p c j m", c=16, j=4),
                    bitv[:, :, 2:16:4, :],
                )
                for g in range(1, 8):
                    nc.sync.dma_start(I16E[16 * g : 16 * (g + 1), :], I16E[0:16, :])
                    nc.sync.dma_start(I16O[16 * g : 16 * (g + 1), :], I16O[0:16, :])

                # ---- phase 1: G = mlp2(F) into FG[64:128], then per-quarter
                # XBAR transpose to token-major and DMA out to the DRAM tables
                # (quarter pipelining overlaps G-mlp, transpose and writes) ----
                tok_v = tokd.ap().rearrange("(r p) e -> p r e", p=P)
                tok2_v = tokd2.ap().rearrange("(r p) e -> p r e", p=P)
                for q in range(4):
                    qs = slice(q * 4096, q * 4096 + 2048)
                    qs2 = slice(q * 4096 + 2048, q * 4096 + 4096)
                    nc.vector.tensor_copy(FG[0:64, qs], FT32[:, qs])
                    nc.scalar.copy(FG[0:64, qs2], FT32[:, qs2])
                    for nt8 in range(8):
                        nt = q * 8 + nt8
                        cols = slice(nt * 512, nt * 512 + 512)
                        g2a_ps = psum.tile([P, 512], f32, tag="mm")
                        nc.tensor.matmul(g2a_ps, W2A[0:64, :], FG[0:64, cols], start=True, stop=True)
                        g2a = wtmp.tile([P, 512], bf16, tag="g2a", bufs=3)
                        nc.scalar.activation(g2a, g2a_ps, RELU, bias=B2A)
                        g2b_ps = psum.tile([P, 512], f32, tag="mm")
                        nc.tensor.matmul(
                            g2b_ps[64:128, :], W2B, g2a, start=True, stop=True,
                            tile_position=(0, 64),
                        )
                        nc.scalar.activation(FG[64:128, cols], g2b_ps[64:128, :], RELU, bias=B2B[64:128, :])
                    STG = wtmp.tile([P, 32, P], bf16, tag="stg", bufs=2, name=f"stg{q}")
                    nc.sync.dma_start_transpose(STG, FG[:, q * 4096 : (q + 1) * 4096])
                    rs = slice(q * 32, q * 32 + 32)
                    eng = nc.sync if q % 2 == 0 else nc.scalar
                    eng2 = nc.scalar if q % 2 == 0 else nc.sync
                    eng.dma_start(tok_v[:, rs, :], STG)
                    eng2.dma_start(tok2_v[:, rs, 0:64], STG[:, :, 64:128])
                    eng2.dma_start(tok2_v[:, rs, 64:128], STG[:, :, 0:64])

                # ---- big weight preps (overlap the gather warm-up) ----
                nat1 = wtmp.tile([P, 4, 512], f32, tag="wnat")
                nc.sync.dma_start(nat1, w1a.ap().rearrange("(ro p) c -> p ro c", p=P))
                for co in range(4):
                    for ro in range(4):
                        pt = psum.tile([P, P], f32, tag="mm")
                        base = co * 128
                        nc.tensor.transpose(pt[0:64, :], nat1[:, ro, base + 64 : base + 128], idf)
                        # transpose-mode MMs must write PSUM partition 0; use a
                        # plain matmul against identity for the partition-64 half
                        nc.tensor.matmul(
                            pt[64:128, :], nat1[:, ro, base : base + 64], idf,
                            start=True, stop=True, tile_position=(0, 64),
                        )
                        nc.vector.tensor_copy(W1[:, co * 512 + ro * 128 : co * 512 + ro * 128 + 128], pt)

                nc.gpsimd.memset(W1Z, 0.0)
                nc.vector.tensor_copy(W1Z[0:64, 0:2048], W1[0:64, :])
                nc.vector.tensor_copy(W1Z[64:128, 2048:4096], W1[64:128, :])

                nat2 = wtmp.tile([P, 4, 512], f32, tag="wnat")
                nc.sync.dma_start(nat2, w1b.ap().rearrange("(ro p) c -> p ro c", p=P))
                for ci in range(4):
                    for mo in range(4):
                        pt = psum.tile([P, P], f32, tag="mm")
                        nc.tensor.transpose(pt, nat2[:, mo, ci * 128 : ci * 128 + 128], idf)
                        nc.vector.tensor_copy(W1B[:, ci * 512 + mo * 128 : ci * 512 + mo * 128 + 128], pt)

                nat3 = wtmp.tile([P, 2, 512], f32, tag="wnat3")
                nc.sync.dma_start(nat3, w3a.ap().rearrange("(ro p) c -> p ro c", p=P))
                for ti in range(4):
                    for mo in range(2):
                        pt = psum.tile([P, P], f32, tag="mm")
                        nc.tensor.transpose(pt, nat3[:, mo, ti * 128 : ti * 128 + 128], idf)
                        nc.vector.tensor_copy(W3A[:, ti * 256 + mo * 128 : ti * 256 + mo * 128 + 128], pt)

                nat4 = wtmp.tile([P, 256], f32, tag="wnat")
                nc.sync.dma_start(nat4, w3b.ap())
                for ti in range(2):
                    pt = psum.tile([P, P], f32, tag="mm")
                    nc.tensor.transpose(pt, nat4[:, ti * 128 : ti * 128 + 128], idf)
                    nc.vector.tensor_copy(W3B[:, ti * 128 : ti * 128 + 128], pt)

                # ---- remaining biases ----
                B1A = const.tile([P, 4], f32)
                nc.sync.dma_start(B1A, b1a.ap().rearrange("(o p) -> p o", p=P))
                B1B = const.tile([P, 4], f32)
                nc.sync.dma_start(B1B, b1b.ap().rearrange("(o p) -> p o", p=P))
                B3A = const.tile([P, 2], f32)
                nc.sync.dma_start(B3A, b3a.ap().rearrange("(o p) -> p o", p=P))
                B3B = const.tile([P, 1], f32)
                nc.sync.dma_start(B3B, b3b.ap()[:, None])

            # ---- phase 2 ----
            with (
                tc.tile_pool(name="gath", bufs=1) as gathp,
                tc.tile_pool(name="work", bufs=1) as workp,
            ):
              gte, gto, tte, tto = {}, {}, {}, {}
              _ordinal = [1]  # FG... pool-DMA ordinal 0 is the W1Z memset? no:
              # ordinal 0 is unused now (memset is not a DMA); first gather
              # ordinal starts at 1 to keep lane/queue alignment with the
              # 8-lane DMASW rotation (queue must equal ordinal mod 4).

              def _gather(dst, tab, idx_ap, n):
                  q = _ordinal[0] % 4 if QMULTI else 0
                  _ordinal[0] += 1
                  nc.gpsimd.dma_gather(
                      dst, tab.ap(), idx_ap,
                      num_idxs=n, num_idxs_reg=n, elem_size=P,
                      transpose=False, single_packet=False, queue_num=q,
                  )

              def issue_gathers(c):
                  gte[c] = gathp.tile([P, 32, P], bf16, tag="gte", bufs=GBUF, name=f"gte{c}")
                  gto[c] = gathp.tile([P, 32, P], bf16, tag="gto", bufs=GBUF, name=f"gto{c}")
                  if c < 3:
                      # warm-up: split into halves so 4 queue streams fill
                      # immediately and the first TT tiles arrive sooner
                      for hh in range(2):
                          _gather(gte[c][:, 16 * hh : 16 * hh + 16, :],
                                  tokd2, I16E[:, c * 256 + 128 * hh : c * 256 + 128 * hh + 128],
                                  NIDX // 2)
                      for hh in range(2):
                          _gather(gto[c][:, 16 * hh : 16 * hh + 16, :],
                                  tokd, I16O[:, c * 256 + 128 * hh : c * 256 + 128 * hh + 128],
                                  NIDX // 2)
                  else:
                      _gather(gte[c][:], tokd2, I16E[:, c * 256 : (c + 1) * 256], NIDX)
                      _gather(gto[c][:], tokd, I16O[:, c * 256 : (c + 1) * 256], NIDX)

              def issue_transposes(c):
                  # PE re-block to channel-major: TT[:, s, p] = GT[p, s, :];
                  # even rows are [G|F] from tokd2 so no parity swap is needed.
                  tte[c] = gathp.tile([P, 32, P], bf16, tag="tte", bufs=TBUF, name=f"tte{c}")
                  tto[c] = gathp.tile([P, 32, P], bf16, tag="tto", bufs=TBUF, name=f"tto{c}")
                  for sb in range(8):
                      pe_ = psum.tile([P, 512], bf16, tag="mm")
                      po_ = psum.tile([P, 512], bf16, tag="mm")
                      for u in range(4):
                          s_ = sb * 4 + u
                          us = slice(u * P, u * P + P)
                          nc.tensor.transpose(pe_[:, us], gte[c][:, s_, :], idb)
                          nc.tensor.transpose(po_[:, us], gto[c][:, s_, :], idb)
                      bs = slice(sb * 4, sb * 4 + 4)
                      if sb % 2 == 0:
                          nc.scalar.copy(tte[c][:, bs, :].rearrange("p a b -> p (a b)"), pe_)
                          nc.vector.tensor_copy(tto[c][:, bs, :].rearrange("p a b -> p (a b)"), po_)
                      else:
                          nc.vector.tensor_copy(tte[c][:, bs, :].rearrange("p a b -> p (a b)"), pe_)
                          nc.scalar.copy(tto[c][:, bs, :].rearrange("p a b -> p (a b)"), po_)

              def issue_compute(c):
                  # within-chunk column i = j*1024 + q, q = ti*16 + p16
                  #   <-> point n = c*1024 + p16*64 + ti
                  # TTE columns are [G|F] (even), TTO are [F|G] (odd).
                  GEv = tte[c].rearrange("p a b -> p (a b)")
                  GOv = tto[c].rearrange("p a b -> p (a b)")

                  # layer 1a for both halves first, so the TT tiles are fully
                  # consumed early and the next chunk's transposes can overlap
                  # the 1b/3a/3b tail.
                  zr = {}
                  for h in range(2):
                      for o in range(4):
                          z1 = psum.tile([P, 512], f32, tag="mm")
                          for j in range(4):
                              cs = slice(j * 1024 + h * 512, j * 1024 + h * 512 + 512)
                              nc.tensor.matmul(
                                  z1, W1Z[:, j * 512 + o * 128 : j * 512 + o * 128 + 128],
                                  GOv[:, cs], start=(j == 0), stop=False,
                              )
                              nc.tensor.matmul(
                                  z1, W1Z[:, 2048 + j * 512 + o * 128 : 2048 + j * 512 + o * 128 + 128],
                                  GEv[:, cs], start=False, stop=(j == 3),
                              )
                          t = workp.tile([P, 512], bf16, tag="zr", bufs=10, name=f"zr{h}{o}")
                          nc.scalar.activation(t, z1, RELU, bias=B1A[:, o : o + 1])
                          zr[h, o] = t
                  # S = Gg + L ... but L is not ready yet; Gg slices are read
                  # into S-staging via the adds below after 1b produces L.
                  lr = {}
                  for h in range(2):
                      for o in range(4):
                          lps = psum.tile([P, 512], f32, tag="mm")
                          for ci in range(4):
                              nc.tensor.matmul(
                                  lps, W1B[:, ci * 512 + o * 128 : ci * 512 + o * 128 + 128],
                                  zr[h, ci], start=(ci == 0), stop=(ci == 3),
                              )
                          t = workp.tile([P, 512], bf16, tag="lr", bufs=10, name=f"lr{h}{o}")
                          nc.scalar.activation(t, lps, RELU, bias=B1B[:, o : o + 1])
                          lr[h, o] = t
                  # S = Gg + L  (even-k G sits on TTE[64:128]: cross-partition)
                  S = {}
                  for h in range(2):
                      for t_ in range(4):
                          cs = slice(t_ * 1024 + h * 512, t_ * 1024 + h * 512 + 512)
                          st = workp.tile([P, 512], bf16, tag="s", bufs=10, name=f"s{h}{t_}")
                          nc.vector.tensor_tensor(
                              st[0:64], GEv[0:64, cs], lr[h, t_][0:64], ADD
                          )
                          nc.vector.tensor_tensor(
                              st[64:128], GOv[64:128, cs], lr[h, t_][64:128], ADD
                          )
                          S[h, t_] = st
                  zr3 = workp.tile([P, 2, CH], bf16, tag="zr3", bufs=3)
                  for h in range(2):
                      hs = slice(h * 512, h * 512 + 512)
                      for o3 in range(2):
                          z3 = psum.tile([P, 512], f32, tag="mm")
                          for t_ in range(4):
                              nc.tensor.matmul(
                                  z3, W3A[:, t_ * 256 + o3 * 128 : t_ * 256 + o3 * 128 + 128],
                                  S[h, t_], start=(t_ == 0), stop=(t_ == 3),
                              )
                          nc.scalar.activation(
                              zr3[:, o3, hs], z3, RELU, bias=B3A[:, o3 : o3 + 1],
                          )
                  # layer 3b: un-permute q = ti*16 + p16 -> n = p16*64 + ti via rhs AP
                  zr3v = zr3.rearrange("p t (ti sixteen) -> p t sixteen ti", sixteen=16)
                  for v in range(2):
                      ops = psum.tile([P, 512], f32, tag="mm")
                      for t_ in range(2):
                          rhs = zr3v[:, t_, 8 * v : 8 * v + 8, :]
                          nc.tensor.matmul(
                              ops, W3B[:, t_ * 128 : t_ * 128 + 128], rhs,
                              start=(t_ == 0), stop=(t_ == 1),
                          )
                      osb = workp.tile([P, 512], f32, tag="osb", bufs=3, name=f"osb{v}")
                      nc.scalar.activation(osb, ops, RELU, bias=B3B)
                      nc.scalar.dma_start(out.ap()[:, c * 1024 + v * 512 : c * 1024 + v * 512 + 512], osb)

              for c in range(min(GBUF, NCH)):
                  issue_gathers(c)
              for c in range(min(TBUF - 1, NCH)):
                  issue_transposes(c)
              for c in range(NCH):
                  issue_compute(c)
                  if c + TBUF - 1 < NCH:
                      issue_transposes(c + TBUF - 1)
                  if c + GBUF < NCH:
                      issue_gathers(c + GBUF)

    nc.compile()
    _cache["nc"] = nc
    return nc


def kernel(**inputs):
    from concourse import bass_utils

    nc = _build()
    feature = np.ascontiguousarray(inputs["feature"], dtype=np.float32)
    idx = np.ascontiguousarray(inputs["idx"], dtype=np.int32)
    weights = {
        k: np.ascontiguousarray(np.asarray(inputs[k]), dtype=np.float32)
        for k in ("w1a", "b1a", "w1b", "b1b", "w2a", "b2a", "w2b", "b2b",
                  "w3a", "b3a", "w3b", "b3b")
    }
    in_maps = []
    for b in range(8):
        m = {"feature": feature[b], "idx": idx[b]}
        m.update(weights)
        in_maps.append(m)
    res = bass_utils.run_bass_kernel_spmd(nc, in_maps, core_ids=list(range(8)))
    return np.stack([res.results[b]["out"] for b in range(8)]).astype(np.float32)


# revision 28
# speedup vs baseline: 1.0046x; 1.0004x over previous
"""PointConv2 Trainium2 Bass kernel.

Data-parallel over B=8 across 8 NeuronCores (one batch element per core).

Per-core computation (feature F [64,16384] f32, idx [16384,8] i32 -> out [128,16384] f32):
  G  = relu(w2b @ relu(w2a @ F + b2a) + b2b)                 [64, N]
  gf[k*64+c, n]  = F[c, idx[n,k]]
  Gg[k*64+c', n] = G[c', idx[n,k]]
  L  = relu(w1b @ relu(w1a @ gf + b1a) + b1b)                [512, N]
  out = relu(w3b @ relu(w3a @ (Gg + L) + b3a) + b3b)         [128, N]

Strategy (v2 — multi-queue gathers, j-major contiguous layout):
  * All matmuls bf16 (fp32 PSUM accumulate).
  * Phase 1: cast feature to bf16 into FG[0:64], compute G into FG[64:128]
    (matmul col-positioned at partitions 64-127), PE-transpose FG into a
    token-major SBUF table TOK (one 384B row per point: [F|G|F]).
  * Phase 1b: PE-transpose FG and stream it to a DRAM token table TOKD
    [N, 128] bf16 (one 256B [F|G] row per token).
  * Phase 2: per chunk of 1024 points, two dma_gather(transpose=False,
    DRAM-source) calls fetch the 8 neighbor rows per point (4 even k's and
    4 odd k's).  Gathers are issued round-robin over 4 SWDGE queues with
    several chunks in flight; the Q7 cluster generates descriptors for 4
    gathers concurrently (~2ns/item vs ~8ns single-queue; the transposed
    SBUF-source gather corrupts data when run multi-queue, the row-gather
    does not).  A PE transpose stage then re-blocks the gathered rows to
    channel-major columns, swapping halves for even k so odd-k F lands on
    partitions 0-63 and even-k F on 64-127 (balanced PE row groups, and
    partition-aligned Gg+L adds).
  * Gather idx lists are pre-arranged j-major: within a chunk, output
    column i = j*1024 + q (j = k-pair, q = ti*16 + p enumerates points
    n = c*1024 + p*64 + ti).  All layer-1a/1b/3a matmul rhs operands and
    the Gg+L adds are then fully contiguous 512-column slices.  The final
    3b matmul un-permutes q -> n via a strided rhs access pattern so the
    output DMA is contiguous.
"""

import os

import numpy as np

STAGE = int(os.environ.get("PC_STAGE", "9"))
QMULTI = int(os.environ.get("PC_QMULTI", "1"))

N = 16384
P = 128
CH = 1024          # points per gather chunk
NCH = N // CH      # 16
NIDX = CH * 4      # idxs per gather call (4 even or 4 odd k's)
ROWB = 384         # bytes per token row in TOK ([F|G|F] bf16)
GBUF = 4           # gather tiles in flight per parity
TBUF = 3           # transposed tiles in flight per parity

_cache = {}


def _build():
    if "nc" in _cache:
        return _cache["nc"]

    import concourse.bass as bass
    import concourse.mybir as mybir
    import concourse.tile as tile
    from concourse.bacc import Bacc
    from concourse.masks import make_identity

    f32 = mybir.dt.float32
    bf16 = mybir.dt.bfloat16
    i32 = mybir.dt.int32
    i16 = mybir.dt.int16
    RELU = mybir.ActivationFunctionType.Relu
    ADD = mybir.AluOpType.add

    nc = Bacc("TRN2", target_bir_lowering=False, debug=False, num_devices=8,
              num_swdge_queues=4)

    feature = nc.dram_tensor("feature", [64, N], f32, kind="ExternalInput")
    idx = nc.dram_tensor("idx", [N, 8], i32, kind="ExternalInput")
    w1a = nc.dram_tensor("w1a", [512, 512], f32, kind="ExternalInput")
    b1a = nc.dram_tensor("b1a", [512], f32, kind="ExternalInput")
    w1b = nc.dram_tensor("w1b", [512, 512], f32, kind="ExternalInput")
    b1b = nc.dram_tensor("b1b", [512], f32, kind="ExternalInput")
    w2a = nc.dram_tensor("w2a", [128, 64], f32, kind="ExternalInput")
    b2a = nc.dram_tensor("b2a", [128], f32, kind="ExternalInput")
    w2b = nc.dram_tensor("w2b", [64, 128], f32, kind="ExternalInput")
    b2b = nc.dram_tensor("b2b", [64], f32, kind="ExternalInput")
    w3a = nc.dram_tensor("w3a", [256, 512], f32, kind="ExternalInput")
    b3a = nc.dram_tensor("b3a", [256], f32, kind="ExternalInput")
    w3b = nc.dram_tensor("w3b", [128, 256], f32, kind="ExternalInput")
    b3b = nc.dram_tensor("b3b", [128], f32, kind="ExternalInput")
    out = nc.dram_tensor("out", [128, N], f32, kind="ExternalOutput")
    tokd = nc.dram_tensor("tokd", [N, P], bf16, kind="Internal")
    tokd2 = nc.dram_tensor("tokd2", [N, P], bf16, kind="Internal")

    with tile.TileContext(nc) as tc:
        with (
            tc.tile_pool(name="const", bufs=1) as const,
            tc.tile_pool(name="tok", bufs=1) as tokp,
            tc.tile_pool(name="idxp", bufs=1) as idxp,
            tc.tile_pool(name="psum", bufs=8, space="PSUM") as psum,
        ):
            idf = const.tile([P, P], f32)
            make_identity(nc, idf)
            idb = const.tile([P, P], bf16)
            make_identity(nc, idb)

            # ---- weights to bf16 lhsT layouts (via PE transpose) ----
            W1 = const.tile([P, 2048], bf16)    # [c][j*512+m]: p<64 -> w1a[m,(2j+1)*64+p], p>=64 -> w1a[m,(2j)*64+(p-64)]
            # zero-padded K=128 variants: odd-k weights (rows 0-63 live) and
            # even-k weights (rows 64-127 live)
            W1Z = const.tile([P, 4096], bf16)
            W1B = const.tile([P, 2048], bf16)   # [p][ci*512+m] = w1b[m, ci*128+p]
            W3A = const.tile([P, 1024], bf16)   # [p][ti*256+m] = w3a[m, ti*128+p]
            W3B = const.tile([P, 256], bf16)    # [p][ti*128+m] = w3b[m, ti*128+p]
            W2A = const.tile([P, 128], bf16)    # [c][m] = w2a[m, c] on partitions 0-63
            W2B = const.tile([P, 64], bf16)     # [p][m] = w2b[m, p]

            with tc.tile_pool(name="wtmp", bufs=2) as wtmp:
                # feature loads as f32 via HWDGE (fast), cast to bf16 on
                # vector/scalar per quarter (overlaps G-mlp pipeline)
                FG = wtmp.tile([P, N], bf16, tag="fg", bufs=1)
                FT32 = wtmp.tile([64, N], f32, tag="ft32", bufs=1)
                for q in range(4):
                    nc.sync.dma_start(FT32[:, q * 4096 : (q + 1) * 4096],
                                      feature.ap()[:, q * 4096 : (q + 1) * 4096])


                # ---- small W2 preps first: G-mlp + token tables are the
                # critical chain to the first gather; big weight preps follow
                # and overlap the gather warm-up ----
                nat5 = wtmp.tile([P, 64], f32, tag="wnat5", bufs=1)
                nc.sync.dma_start(nat5, w2a.ap())
                nat6 = wtmp.tile([64, 128], f32, tag="wnat6", bufs=1)
                nc.sync.dma_start(nat6, w2b.ap())
                B2A = const.tile([P, 1], f32)
                nc.sync.dma_start(B2A, b2a.ap()[:, None])
                B2B = const.tile([P, 1], f32)
                nc.sync.dma_start(B2B[64:128, :], b2b.ap()[:, None])
                pt = psum.tile([P, P], f32, tag="mm")
                nc.tensor.transpose(pt[0:64, :], nat5, idf)
                nc.vector.tensor_copy(W2A[0:64, :], pt[0:64, :])
                pt = psum.tile([P, P], f32, tag="mm")
                nc.tensor.transpose(pt[:, 0:64], nat6, idf[0:64, 0:64])
                nc.vector.tensor_copy(W2B, pt[:, 0:64])

                # ---- idx prep (j-major wrapped lists) ----
                # L32[p][c][(ti k)] = idx[c*1024 + p*64 + ti, k]; 2KB runs.
                # Within-chunk gather column becomes i = j*1024 + q with
                # q = ti*16 + p <-> point n = c*1024 + p*64 + ti.
                L32 = wtmp.tile([16, 16, 512], i32, tag="i32", bufs=1)
                nc.sync.dma_start(
                    L32, idx.ap().rearrange("(c p ti) k -> p c (ti k)", c=16, p=16)
                )
                I16E = idxp.tile([P, 4096], i16)
                I16O = idxp.tile([P, 4096], i16)
                bit = L32[:].bitcast(i16)  # [16, 16, 1024 (ti k two)]
                # I16E[p][c*256 + j*64 + ti] = bit[p][c][ti*16 + 4j]
                bitv = bit.rearrange("p c (ti sixteen) -> p c sixteen ti", sixteen=16)
                nc.vector.tensor_copy(
                    I16E[0:16, :].rearrange("p (c j m) -> p c j m", c=16, j=4),
                    bitv[:, :, 0:16:4, :],
                )
                nc.vector.tensor_copy(
                    I16O[0:16, :].rearrange("p (c j m) -> p c j m", c=16, j=4),
                    bitv[:, :, 2:16:4, :],
                )
                for g in range(1, 8):
                    nc.sync.dma_start(I16E[16 * g : 16 * (g + 1), :], I16E[0:16, :])
                    nc.sync.dma_start(I16O[16 * g : 16 * (g + 1), :], I16O[0:16, :])

                # ---- phase 1: G = mlp2(F) into FG[64:128], then per-quarter
                # XBAR transpose to token-major and DMA out to the DRAM tables
                # (quarter pipelining overlaps G-mlp, transpose and writes) ----
                tok_v = tokd.ap().rearrange("(r p) e -> p r e", p=P)
                tok2_v = tokd2.ap().rearrange("(r p) e -> p r e", p=P)
                for q in range(4):
                    qs = slice(q * 4096, q * 4096 + 2048)
                    qs2 = slice(q * 4096 + 2048, q * 4096 + 4096)
                    nc.vector.tensor_copy(FG[0:64, qs], FT32[:, qs])
                    nc.scalar.copy(FG[0:64, qs2], FT32[:, qs2])
                    for nt8 in range(8):
                        nt = q * 8 + nt8
                        cols = slice(nt * 512, nt * 512 + 512)
                        g2a_ps = psum.tile([P, 512], f32, tag="mm")
                        nc.tensor.matmul(g2a_ps, W2A[0:64, :], FG[0:64, cols], start=True, stop=True)
                        g2a = wtmp.tile([P, 512], bf16, tag="g2a", bufs=3)
                        nc.scalar.activation(g2a, g2a_ps, RELU, bias=B2A)
                        g2b_ps = psum.tile([P, 512], f32, tag="mm")
                        nc.tensor.matmul(
                            g2b_ps[64:128, :], W2B, g2a, start=True, stop=True,
                            tile_position=(0, 64),
                        )
                        nc.scalar.activation(FG[64:128, cols], g2b_ps[64:128, :], RELU, bias=B2B[64:128, :])
                    STG = wtmp.tile([P, 32, P], bf16, tag="stg", bufs=2, name=f"stg{q}")
                    nc.sync.dma_start_transpose(STG, FG[:, q * 4096 : (q + 1) * 4096])
                    rs = slice(q * 32, q * 32 + 32)
                    eng = nc.sync if q % 2 == 0 else nc.scalar
                    eng2 = nc.scalar if q % 2 == 0 else nc.sync
                    eng.dma_start(tok_v[:, rs, :], STG)
                    eng2.dma_start(tok2_v[:, rs, 0:64], STG[:, :, 64:128])
                    eng2.dma_start(tok2_v[:, rs, 64:128], STG[:, :, 0:64])

                # ---- big weight preps (overlap the gather warm-up) ----
                nat1 = wtmp.tile([P, 4, 512], f32, tag="wnat")
                nc.sync.dma_start(nat1, w1a.ap().rearrange("(ro p) c -> p ro c", p=P))
                for co in range(4):
                    for ro in range(4):
                        pt = psum.tile([P, P], f32, tag="mm")
                        base = co * 128
                        nc.tensor.transpose(pt[0:64, :], nat1[:, ro, base + 64 : base + 128], idf)
                        # transpose-mode MMs must write PSUM partition 0; use a
                        # plain matmul against identity for the partition-64 half
                        nc.tensor.matmul(
                            pt[64:128, :], nat1[:, ro, base : base + 64], idf,
                            start=True, stop=True, tile_position=(0, 64),
                        )
                        nc.vector.tensor_copy(W1[:, co * 512 + ro * 128 : co * 512 + ro * 128 + 128], pt)

                nc.gpsimd.memset(W1Z, 0.0)
                nc.vector.tensor_copy(W1Z[0:64, 0:2048], W1[0:64, :])
                nc.vector.tensor_copy(W1Z[64:128, 2048:4096], W1[64:128, :])

                nat2 = wtmp.tile([P, 4, 512], f32, tag="wnat")
                nc.sync.dma_start(nat2, w1b.ap().rearrange("(ro p) c -> p ro c", p=P))
                for ci in range(4):
                    for mo in range(4):
                        pt = psum.tile([P, P], f32, tag="mm")
                        nc.tensor.transpose(pt, nat2[:, mo, ci * 128 : ci * 128 + 128], idf)
                        nc.vector.tensor_copy(W1B[:, ci * 512 + mo * 128 : ci * 512 + mo * 128 + 128], pt)

                nat3 = wtmp.tile([P, 2, 512], f32, tag="wnat3")
                nc.sync.dma_start(nat3, w3a.ap().rearrange("(ro p) c -> p ro c", p=P))
                for ti in range(4):
                    for mo in range(2):
                        pt = psum.tile([P, P], f32, tag="mm")
                        nc.tensor.transpose(pt, nat3[:, mo, ti * 128 : ti * 128 + 128], idf)
                        nc.vector.tensor_copy(W3A[:, ti * 256 + mo * 128 : ti * 256 + mo * 128 + 128], pt)

                nat4 = wtmp.tile([P, 256], f32, tag="wnat")
                nc.sync.dma_start(nat4, w3b.ap())
                for ti in range(2):
                    pt = psum.tile([P, P], f32, tag="mm")
                    nc.tensor.transpose(pt, nat4[:, ti * 128 : ti * 128 + 128], idf)
                    nc.vector.tensor_copy(W3B[:, ti * 128 : ti * 128 + 128], pt)

                # ---- remaining biases ----
                B1A = const.tile([P, 4], f32)
                nc.sync.dma_start(B1A, b1a.ap().rearrange("(o p) -> p o", p=P))
                B1B = const.tile([P, 4], f32)
                nc.sync.dma_start(B1B, b1b.ap().rearrange("(o p) -> p o", p=P))
                B3A = const.tile([P, 2], f32)
                nc.sync.dma_start(B3A, b3a.ap().rearrange("(o p) -> p o", p=P))
                B3B = const.tile([P, 1], f32)
                nc.sync.dma_start(B3B, b3b.ap()[:, None])

            # ---- phase 2 ----
            with (
                tc.tile_pool(name="gath", bufs=1) as gathp,
                tc.tile_pool(name="work", bufs=1) as workp,
            ):
              gte, gto, tte, tto = {}, {}, {}, {}
              _ordinal = [1]  # FG... pool-DMA ordinal 0 is the W1Z memset? no:
              # ordinal 0 is unused now (memset is not a DMA); first gather
              # ordinal starts at 1 to keep lane/queue alignment with the
              # 8-lane DMASW rotation (queue must equal ordinal mod 4).

              def _gather(dst, tab, idx_ap, n):
                  q = _ordinal[0] % 4 if QMULTI else 0
                  _ordinal[0] += 1
                  nc.gpsimd.dma_gather(
                      dst, tab.ap(), idx_ap,
                      num_idxs=n, num_idxs_reg=n, elem_size=P,
                      transpose=False, single_packet=False, queue_num=q,
                  )

              def issue_gathers(c):
                  gte[c] = gathp.tile([P, 32, P], bf16, tag="gte", bufs=GBUF, name=f"gte{c}")
                  gto[c] = gathp.tile([P, 32, P], bf16, tag="gto", bufs=GBUF, name=f"gto{c}")
                  if c < 3:
                      # warm-up: split into halves so 4 queue streams fill
                      # immediately and the first TT tiles arrive sooner
                      for hh in range(2):
                          _gather(gte[c][:, 16 * hh : 16 * hh + 16, :],
                                  tokd2, I16E[:, c * 256 + 128 * hh : c * 256 + 128 * hh + 128],
                                  NIDX // 2)
                      for hh in range(2):
                          _gather(gto[c][:, 16 * hh : 16 * hh + 16, :],
                                  tokd, I16O[:, c * 256 + 128 * hh : c * 256 + 128 * hh + 128],
                                  NIDX // 2)
                  else:
                      _gather(gte[c][:], tokd2, I16E[:, c * 256 : (c + 1) * 256], NIDX)
                      _gather(gto[c][:], tokd, I16O[:, c * 256 : (c + 1) * 256], NIDX)

              def issue_transposes(c):
                  # PE re-block to channel-major: TT[:, s, p] = GT[p, s, :];
                  # even rows are [G|F] from tokd2 so no parity swap is needed.
                  tte[c] = gathp.tile([P, 32, P], bf16, tag="tte", bufs=TBUF, name=f"tte{c}")
                  tto[c] = gathp.tile([P, 32, P], bf16, tag="tto", bufs=TBUF, name=f"tto{c}")
                  for sb in range(8):
                      pe_ = psum.tile([P, 512], bf16, tag="mm")
                      po_ = psum.tile([P, 512], bf16, tag="mm")
                      for u in range(4):
                          s_ = sb * 4 + u
                          us = slice(u * P, u * P + P)
                          nc.tensor.transpose(pe_[:, us], gte[c][:, s_, :], idb)
                          nc.tensor.transpose(po_[:, us], gto[c][:, s_, :], idb)
                      bs = slice(sb * 4, sb * 4 + 4)
                      nc.vector.tensor_copy(tte[c][:, bs, :].rearrange("p a b -> p (a b)"), pe_)
                      nc.vector.tensor_copy(tto[c][:, bs, :].rearrange("p a b -> p (a b)"), po_)

              def issue_compute(c):
                  # within-chunk column i = j*1024 + q, q = ti*16 + p16
                  #   <-> point n = c*1024 + p16*64 + ti
                  # TTE columns are [G|F] (even), TTO are [F|G] (odd).
                  GEv = tte[c].rearrange("p a b -> p (a b)")
                  GOv = tto[c].rearrange("p a b -> p (a b)")

                  # layer 1a for both halves first, so the TT tiles are fully
                  # consumed early and the next chunk's transposes can overlap
                  # the 1b/3a/3b tail.
                  zr = {}
                  for h in range(2):
                      for o in range(4):
                          z1 = psum.tile([P, 512], f32, tag="mm")
                          for j in range(4):
                              cs = slice(j * 1024 + h * 512, j * 1024 + h * 512 + 512)
                              nc.tensor.matmul(
                                  z1, W1Z[:, j * 512 + o * 128 : j * 512 + o * 128 + 128],
                                  GOv[:, cs], start=(j == 0), stop=False,
                              )
                              nc.tensor.matmul(
                                  z1, W1Z[:, 2048 + j * 512 + o * 128 : 2048 + j * 512 + o * 128 + 128],
                                  GEv[:, cs], start=False, stop=(j == 3),
                              )
                          t = workp.tile([P, 512], bf16, tag="zr", bufs=10, name=f"zr{h}{o}")
                          nc.scalar.activation(t, z1, RELU, bias=B1A[:, o : o + 1])
                          zr[h, o] = t
                  # S = Gg + L ... but L is not ready yet; Gg slices are read
                  # into S-staging via the adds below after 1b produces L.
                  lr = {}
                  for h in range(2):
                      for o in range(4):
                          lps = psum.tile([P, 512], f32, tag="mm")
                          for ci in range(4):
                              nc.tensor.matmul(
                                  lps, W1B[:, ci * 512 + o * 128 : ci * 512 + o * 128 + 128],
                                  zr[h, ci], start=(ci == 0), stop=(ci == 3),
                              )
                          t = workp.tile([P, 512], bf16, tag="lr", bufs=10, name=f"lr{h}{o}")
                          nc.scalar.activation(t, lps, RELU, bias=B1B[:, o : o + 1])
                          lr[h, o] = t
                  # S = Gg + L  (even-k G sits on TTE[64:128]: cross-partition)
                  S = {}
                  for h in range(2):
                      for t_ in range(4):
                          cs = slice(t_ * 1024 + h * 512, t_ * 1024 + h * 512 + 512)
                          st = workp.tile([P, 512], bf16, tag="s", bufs=10, name=f"s{h}{t_}")
                          nc.vector.tensor_tensor(
                              st[0:64], GEv[0:64, cs], lr[h, t_][0:64], ADD
                          )
                          nc.vector.tensor_tensor(
                              st[64:128], GOv[64:128, cs], lr[h, t_][64:128], ADD
                          )
                          S[h, t_] = st
                  zr3 = workp.tile([P, 2, CH], bf16, tag="zr3", bufs=3)
                  for h in range(2):
                      hs = slice(h * 512, h * 512 + 512)
                      for o3 in range(2):
                          z3 = psum.tile([P, 512], f32, tag="mm")
                          for t_ in range(4):
                              nc.tensor.matmul(
                                  z3, W3A[:, t_ * 256 + o3 * 128 : t_ * 256 + o3 * 128 + 128],
                                  S[h, t_], start=(t_ == 0), stop=(t_ == 3),
                              )
                          nc.scalar.activation(
                              zr3[:, o3, hs], z3, RELU, bias=B3A[:, o3 : o3 + 1],
                          )
                  # layer 3b: un-permute q = ti*16 + p16 -> n = p16*64 + ti via rhs AP
                  zr3v = zr3.rearrange("p t (ti sixteen) -> p t sixteen ti", sixteen=16)
                  for v in range(2):
                      ops = psum.tile([P, 512], f32, tag="mm")
                      for t_ in range(2):
                          rhs = zr3v[:, t_, 8 * v : 8 * v + 8, :]
                          nc.tensor.matmul(
                              ops, W3B[:, t_ * 128 : t_ * 128 + 128], rhs,
                              start=(t_ == 0), stop=(t_ == 1),
                          )
                      osb = workp.tile([P, 512], f32, tag="osb", bufs=3, name=f"osb{v}")
                      nc.scalar.activation(osb, ops, RELU, bias=B3B)
                      nc.sync.dma_start(out.ap()[:, c * 1024 + v * 512 : c * 1024 + v * 512 + 512], osb)

              for c in range(min(GBUF, NCH)):
                  issue_gathers(c)
              for c in range(min(TBUF - 1, NCH)):
                  issue_transposes(c)
              for c in range(NCH):
                  issue_compute(c)
                  if c + TBUF - 1 < NCH:
                      issue_transposes(c + TBUF - 1)
                  if c + GBUF < NCH:
                      issue_gathers(c + GBUF)

    nc.compile()
    _cache["nc"] = nc
    return nc


def kernel(**inputs):
    from concourse import bass_utils

    nc = _build()
    feature = np.ascontiguousarray(inputs["feature"], dtype=np.float32)
    idx = np.ascontiguousarray(inputs["idx"], dtype=np.int32)
    weights = {
        k: np.ascontiguousarray(np.asarray(inputs[k]), dtype=np.float32)
        for k in ("w1a", "b1a", "w1b", "b1b", "w2a", "b2a", "w2b", "b2b",
                  "w3a", "b3a", "w3b", "b3b")
    }
    in_maps = []
    for b in range(8):
        m = {"feature": feature[b], "idx": idx[b]}
        m.update(weights)
        in_maps.append(m)
    res = bass_utils.run_bass_kernel_spmd(nc, in_maps, core_ids=list(range(8)))
    return np.stack([res.results[b]["out"] for b in range(8)]).astype(np.float32)
